# revision 15
# baseline (speedup 1.0000x reference)
"""Distributed Bass kernel for nn_AttentionLayer (B=2,S=2048,D=1024,H=16,DK=DV=64) on 8 TRN2 cores.

Sharding: core c handles batch c//4 and heads [(c%4)*4, (c%4)*4+4) (Megatron
column-sharded QKV).  Attention is computed with scores *transposed* ([k, q]
tiles, k on partitions) so the context matmul needs no on-chip transposes;
softmax row-sums come from a ones-column appended to V.  The attention
probability matrix is written to DRAM as attn^T per head (host re-transposes
during unshard).  The fc layer is token-parallel: a masked 8-core AllToAll
exchanges per-head context slices so each core computes fc+residual+LayerNorm
for its own 512-token slice of the full output.
"""
import sys

sys.path.insert(0, "/opt/trn_rl_repo")

import numpy as np
import ml_dtypes

B, S, D, H, DK, DV = 2, 2048, 1024, 16, 64, 64
NCORES = 8
HLOC = 4          # heads per core
TOK = 512         # tokens per core for the fc/LN output slice
EPS = 1e-6

_cache = {}


def _build():
    import concourse.bacc as bacc
    import concourse.tile as tile
    import concourse.mybir as mybir

    bf16 = mybir.dt.bfloat16
    f32 = mybir.dt.float32
    Act = mybir.ActivationFunctionType
    Alu = mybir.AluOpType

    nc = bacc.Bacc("TRN2", target_bir_lowering=False, debug=False,
                   num_devices=NCORES)

    ipT_ext = nc.dram_tensor("ipT", [D, S], bf16, kind="ExternalInput")
    ipres_ext = nc.dram_tensor("ip_res", [TOK, D], f32, kind="ExternalInput")
    wq_ext = nc.dram_tensor("wq", [D, HLOC * DK], bf16, kind="ExternalInput")
    wk_ext = nc.dram_tensor("wk", [D, HLOC * DK], bf16, kind="ExternalInput")
    wv_ext = nc.dram_tensor("wv", [D, HLOC * DV], bf16, kind="ExternalInput")
    fcw_ext = nc.dram_tensor("fc_w", [D, D], bf16, kind="ExternalInput")
    fcb_ext = nc.dram_tensor("fc_b_bc", [128, D], f32, kind="ExternalInput")
    lng_ext = nc.dram_tensor("ln_g_bc", [128, D], f32, kind="ExternalInput")
    lnb_ext = nc.dram_tensor("ln_b_bc", [128, D], f32, kind="ExternalInput")
    mask_ext = nc.dram_tensor("mask", [128, NCORES], bf16, kind="ExternalInput")

    attn_ext = nc.dram_tensor("attn_out", [HLOC, S, S], bf16, kind="ExternalOutput")
    y_ext = nc.dram_tensor("y_out", [TOK, D], f32, kind="ExternalOutput")

    with tile.TileContext(nc) as tc:
        with tc.tile_pool(name="const", bufs=1) as cpool, \
             tc.tile_pool(name="qkv", bufs=1) as qkvpool, \
             tc.tile_pool(name="dram", bufs=2, space="DRAM") as dpool:
            # ---- constant loads ----
            wq_t = cpool.tile([128, 8, HLOC * DK], bf16, name="wq_t")
            wk_t = cpool.tile([128, 8, HLOC * DK], bf16, name="wk_t")
            wv_t = cpool.tile([128, 8, HLOC * DV], bf16, name="wv_t")
            fcw_t = cpool.tile([128, 8, D], bf16, name="fcw_t")
            mask_t = cpool.tile([128, NCORES], bf16, name="mask_t")
            nc.sync.dma_start(wq_t[:], wq_ext.rearrange("(o i) c -> i o c", i=128))
            nc.sync.dma_start(wk_t[:], wk_ext.rearrange("(o i) c -> i o c", i=128))
            nc.sync.dma_start(wv_t[:], wv_ext.rearrange("(o i) c -> i o c", i=128))
            nc.sync.dma_start(mask_t[:], mask_ext[:])
            nc.sync.dma_start(fcw_t[:], fcw_ext.rearrange("(o i) c -> i o c", i=128))

            # qT/kT: channel-major per head-pair [128 = 2 heads x 64ch, S]
            qT = [qkvpool.tile([128, S], bf16, name=f"qT{p}") for p in range(2)]
            kT = [qkvpool.tile([128, S], bf16, name=f"kT{p}") for p in range(2)]
            # v~: token-major per head with ones column [128, 16, 65]
            v_t = [qkvpool.tile([128, 16, DV + 1], bf16, name=f"v{h}")
                   for h in range(HLOC)]
            # per-head context, channel-major: [64 dv partitions, head, token]
            ctxT_loc = qkvpool.tile([64, HLOC, S], bf16, name="ctxT_loc")

            # ---- projections ----
            with tc.tile_pool(name="ipt", bufs=1) as ipool, \
                 tc.tile_pool(name="proj_ps", bufs=2, space="PSUM") as ppsum:
                ipT_t = ipool.tile([128, 8, S], bf16, name="ipT_t")
                nc.sync.dma_start(ipT_t[:], ipT_ext.rearrange("(o i) t -> i o t", i=128))

                for h in range(HLOC):
                    nc.vector.memset(v_t[h][:, :, DV:DV + 1], 1.0)

                for pair in range(2):
                    for w_t, dst in ((wq_t, qT[pair]), (wk_t, kT[pair])):
                        for th in range(2):
                            ps = ppsum.tile([128, 1024], mybir.dt.float32, tag="qk_ps",
                                            name=f"qk_ps_{pair}_{th}")
                            for tq in range(2):
                                for dc in range(8):
                                    nc.tensor.matmul(
                                        ps[:, tq * 512:(tq + 1) * 512],
                                        w_t[:, dc, pair * 128:(pair + 1) * 128],
                                        ipT_t[:, dc, th * 1024 + tq * 512:th * 1024 + (tq + 1) * 512],
                                        start=(dc == 0), stop=(dc == 7))
                            nc.scalar.copy(dst[:, th * 1024:(th + 1) * 1024], ps[:])

                for tb in range(16):
                    psv = ppsum.tile([128, HLOC * DV], mybir.dt.float32, tag="v_ps",
                                     name=f"v_ps_{tb}")
                    for dc in range(8):
                        nc.tensor.matmul(psv[:],
                                         ipT_t[:, dc, tb * 128:(tb + 1) * 128],
                                         wv_t[:, dc, :],
                                         start=(dc == 0), stop=(dc == 7))
                    for h in range(HLOC):
                        nc.vector.tensor_copy(v_t[h][:, tb, 0:DV],
                                              psv[:, h * DV:(h + 1) * DV])

            # ---- attention: units = (head-pair, q-512-chunk), heads interleaved,
            #      1-bank score psums (bufs=3) so PE runs ahead of ACT exp;
            #      per-pair masked A2A fires as soon as the pair's ctx is done
            ctxf_half = []
            with tc.tile_pool(name="comm", bufs=1) as compool, \
                 tc.tile_pool(name="exp", bufs=48) as epool, \
                 tc.tile_pool(name="bc", bufs=3) as bcpool, \
                 tc.tile_pool(name="scA_ps", bufs=3, space="PSUM") as scpsA, \
                 tc.tile_pool(name="scB_ps", bufs=3, space="PSUM") as scpsB, \
                 tc.tile_pool(name="ctxA_ps", bufs=1, space="PSUM") as cxpsA, \
                 tc.tile_pool(name="ctxB_ps", bufs=1, space="PSUM") as cxpsB:
                for pair in range(2):
                    hA, hB = 2 * pair, 2 * pair + 1
                    for qc in range(4):
                        q0 = qc * 512
                        cpsA = cxpsA.tile([DV + 1, 512], mybir.dt.float32,
                                          tag="cpsA", name=f"cpsA_{pair}_{qc}")
                        cpsB = cxpsB.tile([DV + 1, 512], mybir.dt.float32,
                                          tag="cpsB", name=f"cpsB_{pair}_{qc}")
                        ekA, ekB = [], []
                        for kb in range(16):
                            k0 = kb * 128
                            spsA = scpsA.tile([128, 512], mybir.dt.float32,
                                              tag="spsA", name=f"spsA_{pair}_{qc}_{kb}")
                            spsB = scpsB.tile([128, 512], mybir.dt.float32,
                                              tag="spsB", name=f"spsB_{pair}_{qc}_{kb}")
                            nc.tensor.matmul(
                                spsA[:], kT[pair][0:64, k0:k0 + 128],
                                qT[pair][0:64, q0:q0 + 512],
                                start=True, stop=True, tile_position=(0, 0))
                            nc.tensor.matmul(
                                spsB[:], kT[pair][64:128, k0:k0 + 128],
                                qT[pair][64:128, q0:q0 + 512],
                                start=True, stop=True, tile_position=(64, 0))
                            eA = epool.tile([128, 512], bf16, tag="ekb",
                                            name=f"ekA_{pair}_{qc}_{kb}")
                            eB = epool.tile([128, 512], bf16, tag="ekb",
                                            name=f"ekB_{pair}_{qc}_{kb}")
                            ekA.append(eA)
                            ekB.append(eB)
                            nc.scalar.activation(eA[:], spsA[:], Act.Exp, scale=0.125)
                            nc.scalar.activation(eB[:], spsB[:], Act.Exp, scale=0.125)
                            nc.tensor.matmul(cpsA[:], v_t[hA][:, kb, :], eA[:],
                                             start=(kb == 0), stop=(kb == 15))
                            nc.tensor.matmul(cpsB[:], v_t[hB][:, kb, :], eB[:],
                                             start=(kb == 0), stop=(kb == 15))
                        for h, cps, ek in ((hA, cpsA, ekA), (hB, cpsB, ekB)):
                            # reciprocal of sums: spread [1,512] over 128
                            # partitions via DRAM so DVE divide is 4 elem/lane
                            sums_sb = bcpool.tile([DV + 1, 512], mybir.dt.float32,
                                                  tag="sums", name=f"sums_{h}_{qc}")
                            nc.vector.tensor_copy(sums_sb[DV:DV + 1, :],
                                                  cps[DV:DV + 1, :])
                            s_d = dpool.tile([1, 512], mybir.dt.float32, tag="s_d",
                                             name=f"s_d_{h}_{qc}")
                            nc.sync.dma_start(s_d[:], sums_sb[DV:DV + 1, :])
                            sums_sp = bcpool.tile([128, 4], mybir.dt.float32,
                                                  tag="sums_sp", name=f"ssp_{h}_{qc}")
                            nc.sync.dma_start(
                                sums_sp[:],
                                s_d[:].rearrange("o (p f) -> (o p) f", p=128))
                            rec_sp = bcpool.tile([128, 4], mybir.dt.float32,
                                                 tag="rec_sp", name=f"rsp_{h}_{qc}")
                            nc.vector.reciprocal(rec_sp[:], sums_sp[:])
                            rec_bf = bcpool.tile([128, 4], bf16, tag="rec_bf",
                                                 name=f"rbf_{h}_{qc}")
                            nc.vector.tensor_copy(rec_bf[:], rec_sp[:])
                            rb_d = dpool.tile([1, 512], bf16, tag="rb_d",
                                              name=f"rb_d_{h}_{qc}")
                            nc.sync.dma_start(
                                rb_d[:].rearrange("o (p f) -> (o p) f", p=128),
                                rec_bf[:])
                            bcast = bcpool.tile([128, 512], bf16, tag="bcast",
                                                name=f"bcast_{h}_{qc}")
                            nc.sync.dma_start(bcast[:],
                                              rb_d[:].to_broadcast((128, 512)))
                            # normalized context -> ctxT_loc (ch = h*64 + p)
                            cdst = ctxT_loc[:, h, q0:q0 + 512]
                            nc.vector.tensor_copy(cdst, cps[0:DV, :])
                            nc.vector.tensor_tensor(cdst, cdst, bcast[0:DV, :],
                                                    Alu.mult)
                            # normalized attention -> DRAM (bf16; host casts to f32)
                            for kb in range(16):
                                e = ek[kb]
                                if kb % 2 == 0:
                                    nc.vector.tensor_tensor(e[:], e[:], bcast[:],
                                                            Alu.mult)
                                else:
                                    nc.gpsimd.tensor_tensor(e[:], e[:], bcast[:],
                                                            Alu.mult)
                                nc.sync.dma_start(
                                    attn_ext[h, kb * 128:(kb + 1) * 128,
                                             q0:q0 + 512],
                                    e[:])
                    # masked A2A for this pair's 128-channel half
                    staged = compool.tile([64, 2, 4, 2, 512], bf16, tag="staged",
                                          name=f"staged{pair}")
                    csrc = ctxT_loc[:, 2 * pair:2 * pair + 2, :] \
                        .rearrange("p h (c t) -> p c h t", t=512)
                    for d in range(2):
                        nc.vector.tensor_tensor(
                            staged[:, d], csrc[:],
                            mask_t[0:64, d * 4:(d + 1) * 4][:, :, None, None]
                                .to_broadcast((64, 4, 2, 512)),
                            Alu.mult)
                    a_in = dpool.tile([NCORES, 2, 64, 512], bf16,
                                      name=f"a2a_in{pair}")
                    a_out = dpool.tile([NCORES, 2, 64, 512], bf16,
                                       name=f"a2a_out{pair}")
                    nc.sync.dma_start(
                        a_in[:].rearrange("(d c) h p t -> p d c h t", d=2),
                        staged[:])
                    nc.gpsimd.collective_compute(
                        "AllToAll", Alu.bypass,
                        replica_groups=[list(range(NCORES))],
                        ins=[a_in.opt()], outs=[a_out.opt()])
                    rcv_lo = compool.tile([128, 4, 512], bf16, tag="rcvlo",
                                          name=f"rcvlo{pair}")
                    rcv_hi = compool.tile([128, 4, 512], bf16, tag="rcvhi",
                                          name=f"rcvhi{pair}")
                    nc.sync.dma_start(rcv_lo[:],
                                      a_out[0:4].rearrange("sl h p t -> (h p) sl t"))
                    nc.sync.dma_start(rcv_hi[:],
                                      a_out[4:8].rearrange("sl h p t -> (h p) sl t"))
                    cf = qkvpool.tile([128, 4, 512], bf16, name=f"ctxf{pair}")
                    nc.vector.tensor_tensor(cf[:], rcv_lo[:], rcv_hi[:], Alu.add)
                    ctxf_half.append(cf)

            # ---- tail: fc, residual, LayerNorm ----
            with tc.tile_pool(name="tail", bufs=1) as tpool, \
                 tc.tile_pool(name="ln", bufs=4) as lnpool, \
                 tc.tile_pool(name="sq", bufs=2) as sqpool, \
                 tc.tile_pool(name="fc_ps", bufs=2, space="PSUM") as fcpsum:
                ipres_t = tpool.tile([128, 4, D], mybir.dt.float32, name="ipres_t")
                fcb_t = tpool.tile([128, D], mybir.dt.float32, name="fcb_t")
                lng_t = tpool.tile([128, D], mybir.dt.float32, name="lng_t")
                lnb_t = tpool.tile([128, D], mybir.dt.float32, name="lnb_t")
                nc.sync.dma_start(ipres_t[:],
                                  ipres_ext.rearrange("(tb p) d -> p tb d", p=128))
                nc.sync.dma_start(fcb_t[:], fcb_ext[:])
                nc.sync.dma_start(lng_t[:], lng_ext[:])
                nc.sync.dma_start(lnb_t[:], lnb_ext[:])

                x_t = tpool.tile([128, 4, D], mybir.dt.float32, name="x_t")
                y_t = tpool.tile([128, 4, D], mybir.dt.float32, name="y_t")
                for tb in range(4):
                    fps = fcpsum.tile([128, D], mybir.dt.float32, tag="fps",
                                      name=f"fps_{tb}")
                    for n2 in range(2):
                        # global ch chunk cc = sl*2 + s  (s = pair half)
                        for i, (s, sl) in enumerate(
                                [(s, sl) for s in range(2) for sl in range(4)]):
                            nc.tensor.matmul(
                                fps[:, n2 * 512:(n2 + 1) * 512],
                                ctxf_half[s][:, sl, tb * 128:(tb + 1) * 128],
                                fcw_t[:, sl * 2 + s, n2 * 512:(n2 + 1) * 512],
                                start=(i == 0), stop=(i == 7))
                    xs = x_t[:, tb, :]
                    nc.vector.tensor_tensor(xs, fps[:], ipres_t[:, tb, :], Alu.add)
                    nc.vector.tensor_tensor(xs, xs, fcb_t[:], Alu.add)
                    # LayerNorm over D
                    ssum = lnpool.tile([128, 1], mybir.dt.float32, tag="ssum",
                                       name=f"ssum_{tb}")
                    nc.vector.tensor_reduce(ssum[:], xs, mybir.AxisListType.X, Alu.add)
                    nmu = lnpool.tile([128, 1], mybir.dt.float32, tag="nmu",
                                      name=f"nmu_{tb}")
                    nc.vector.tensor_scalar_mul(nmu[:], ssum[:], -1.0 / D)
                    sq = sqpool.tile([128, D], mybir.dt.float32, tag="sq",
                                     name=f"sq_{tb}")
                    ssq = lnpool.tile([128, 1], mybir.dt.float32, tag="ssq",
                                      name=f"ssq_{tb}")
                    nc.scalar.activation(sq[:], xs, Act.Square, bias=nmu[:],
                                         scale=1.0, accum_out=ssq[:])
                    veps = lnpool.tile([128, 1], mybir.dt.float32, tag="veps",
                                       name=f"veps_{tb}")
                    nc.vector.tensor_scalar(veps[:], ssq[:], 1.0 / D, EPS,
                                            Alu.mult, Alu.add)
                    lnv = lnpool.tile([128, 1], mybir.dt.float32, tag="lnv",
                                      name=f"lnv_{tb}")
                    nc.scalar.activation(lnv[:], veps[:], Act.Ln)
                    rstd = lnpool.tile([128, 1], mybir.dt.float32, tag="rstd",
                                       name=f"rstd_{tb}")
                    nc.scalar.activation(rstd[:], lnv[:], Act.Exp, scale=-0.5)
                    ys = y_t[:, tb, :]
                    nc.vector.tensor_scalar(ys, xs, nmu[:], rstd[:],
                                            Alu.add, Alu.mult)
                    nc.vector.tensor_tensor(ys, ys, lng_t[:], Alu.mult)
                    nc.vector.tensor_tensor(ys, ys, lnb_t[:], Alu.add)
                nc.sync.dma_start(y_ext.rearrange("(tb p) d -> p tb d", p=128), y_t[:])

    nc.finalize()
    return nc


def _prep_inputs(ip, wq, wk, wv, fc_w, fc_b, ln_g, ln_b):
    bf = ml_dtypes.bfloat16
    ip = np.asarray(ip, np.float32)
    wq = np.asarray(wq, np.float32)
    wk = np.asarray(wk, np.float32)
    wv = np.asarray(wv, np.float32)
    fc_w = np.asarray(fc_w, np.float32)
    fc_b = np.asarray(fc_b, np.float32)
    ln_g = np.asarray(ln_g, np.float32)
    ln_b = np.asarray(ln_b, np.float32)

    ipT = [np.ascontiguousarray(ip[b].T).astype(bf) for b in range(B)]
    fcw_bf = fc_w.astype(bf)
    fcb_bc = np.ascontiguousarray(np.broadcast_to(fc_b, (128, D))).astype(np.float32)
    lng_bc = np.ascontiguousarray(np.broadcast_to(ln_g, (128, D))).astype(np.float32)
    lnb_bc = np.ascontiguousarray(np.broadcast_to(ln_b, (128, D))).astype(np.float32)

    in_maps = []
    for c in range(NCORES):
        b, g = c // NCORES * 0 + c // 4, c % 4
        cols = slice(g * HLOC * DK, (g + 1) * HLOC * DK)
        mask = np.zeros((128, NCORES), np.float32)
        mask[:, b * 4:(b + 1) * 4] = 1.0
        in_maps.append({
            "ipT": ipT[b],
            "ip_res": np.ascontiguousarray(ip[b, g * TOK:(g + 1) * TOK]),
            "wq": np.ascontiguousarray(wq[:, cols]).astype(bf),
            "wk": np.ascontiguousarray(wk[:, cols]).astype(bf),
            "wv": np.ascontiguousarray(wv[:, cols]).astype(bf),
            "fc_w": fcw_bf,
            "fc_b_bc": fcb_bc,
            "ln_g_bc": lng_bc,
            "ln_b_bc": lnb_bc,
            "mask": mask.astype(bf),
        })
    return in_maps


def _run(in_maps, trace=False):
    from concourse.bass_utils import run_bass_kernel_spmd
    if "nc" not in _cache:
        _cache["nc"] = _build()
    return run_bass_kernel_spmd(_cache["nc"], in_maps,
                                core_ids=list(range(NCORES)), trace=trace)


def kernel(ip, wq, wk, wv, fc_w, fc_b, ln_g, ln_b, _trace=False):
    in_maps = _prep_inputs(ip, wq, wk, wv, fc_w, fc_b, ln_g, ln_b)
    res = _run(in_maps, trace=_trace)

    y = np.empty((B, S, D), np.float32)
    attn = np.empty((B, H, S, S), np.float32)
    for c in range(NCORES):
        b, g = c // 4, c % 4
        r = res.results[c]
        y[b, g * TOK:(g + 1) * TOK] = r["y_out"]
        for hl in range(HLOC):
            attn[b, g * HLOC + hl] = r["attn_out"][hl].T.astype(np.float32)
    if _trace:
        kernel.last_exec_time_ns = res.exec_time_ns
        kernel.last_results = res
    return y, attn


# revision 16
# speedup vs baseline: 1.0151x; 1.0151x over previous
"""Distributed Bass kernel for nn_AttentionLayer (B=2,S=2048,D=1024,H=16,DK=DV=64) on 8 TRN2 cores.

Sharding: core c handles batch c//4 and heads [(c%4)*4, (c%4)*4+4) (Megatron
column-sharded QKV).  Attention is computed with scores *transposed* ([k, q]
tiles, k on partitions) so the context matmul needs no on-chip transposes;
softmax row-sums come from a ones-column appended to V.  The attention
probability matrix is written to DRAM as attn^T per head (host re-transposes
during unshard).  The fc layer is token-parallel: a masked 8-core AllToAll
exchanges per-head context slices so each core computes fc+residual+LayerNorm
for its own 512-token slice of the full output.
"""
import sys

sys.path.insert(0, "/opt/trn_rl_repo")

import numpy as np
import ml_dtypes

B, S, D, H, DK, DV = 2, 2048, 1024, 16, 64, 64
NCORES = 8
HLOC = 4          # heads per core
TOK = 512         # tokens per core for the fc/LN output slice
EPS = 1e-6

_cache = {}


def _build():
    import concourse.bacc as bacc
    import concourse.tile as tile
    import concourse.mybir as mybir

    bf16 = mybir.dt.bfloat16
    f32 = mybir.dt.float32
    Act = mybir.ActivationFunctionType
    Alu = mybir.AluOpType

    nc = bacc.Bacc("TRN2", target_bir_lowering=False, debug=False,
                   num_devices=NCORES)

    ipT_ext = nc.dram_tensor("ipT", [D, S], bf16, kind="ExternalInput")
    ipres_ext = nc.dram_tensor("ip_res", [TOK, D], f32, kind="ExternalInput")
    wq_ext = nc.dram_tensor("wq", [D, HLOC * DK], bf16, kind="ExternalInput")
    wk_ext = nc.dram_tensor("wk", [D, HLOC * DK], bf16, kind="ExternalInput")
    wv_ext = nc.dram_tensor("wv", [D, HLOC * DV], bf16, kind="ExternalInput")
    fcw_ext = nc.dram_tensor("fc_w", [D, D], bf16, kind="ExternalInput")
    fcb_ext = nc.dram_tensor("fc_b_bc", [128, D], f32, kind="ExternalInput")
    lng_ext = nc.dram_tensor("ln_g_bc", [128, D], f32, kind="ExternalInput")
    lnb_ext = nc.dram_tensor("ln_b_bc", [128, D], f32, kind="ExternalInput")
    mask_ext = nc.dram_tensor("mask", [128, NCORES], bf16, kind="ExternalInput")

    attn_ext = nc.dram_tensor("attn_out", [HLOC, S, S], bf16, kind="ExternalOutput")
    y_ext = nc.dram_tensor("y_out", [TOK, D], f32, kind="ExternalOutput")

    with tile.TileContext(nc) as tc:
        with tc.tile_pool(name="const", bufs=1) as cpool, \
             tc.tile_pool(name="qkv", bufs=1) as qkvpool, \
             tc.tile_pool(name="dram", bufs=2, space="DRAM") as dpool:
            # ---- constant loads ----
            wq_t = cpool.tile([128, 8, HLOC * DK], bf16, name="wq_t")
            wk_t = cpool.tile([128, 8, HLOC * DK], bf16, name="wk_t")
            wv_t = cpool.tile([128, 8, HLOC * DV], bf16, name="wv_t")
            fcw_t = cpool.tile([128, 8, D], bf16, name="fcw_t")
            mask_t = cpool.tile([128, NCORES], bf16, name="mask_t")
            nc.sync.dma_start(wq_t[:], wq_ext.rearrange("(o i) c -> i o c", i=128))
            nc.sync.dma_start(wk_t[:], wk_ext.rearrange("(o i) c -> i o c", i=128))
            nc.sync.dma_start(wv_t[:], wv_ext.rearrange("(o i) c -> i o c", i=128))
            nc.sync.dma_start(mask_t[:], mask_ext[:])
            nc.sync.dma_start(fcw_t[:], fcw_ext.rearrange("(o i) c -> i o c", i=128))

            # qT/kT: channel-major per head-pair [128 = 2 heads x 64ch, S]
            qT = [qkvpool.tile([128, S], bf16, name=f"qT{p}") for p in range(2)]
            kT = [qkvpool.tile([128, S], bf16, name=f"kT{p}") for p in range(2)]
            # v~: token-major per head with ones column [128, 16, 65]
            v_t = [qkvpool.tile([128, 16, DV + 1], bf16, name=f"v{h}")
                   for h in range(HLOC)]
            # per-head context, channel-major: [64 dv partitions, head, token]
            ctxT_loc = qkvpool.tile([64, HLOC, S], bf16, name="ctxT_loc")

            # ---- projections ----
            with tc.tile_pool(name="ipt", bufs=1) as ipool, \
                 tc.tile_pool(name="proj_ps", bufs=2, space="PSUM") as ppsum:
                ipT_t = ipool.tile([128, 8, S], bf16, name="ipT_t")
                nc.sync.dma_start(ipT_t[:], ipT_ext.rearrange("(o i) t -> i o t", i=128))

                for h in range(HLOC):
                    nc.vector.memset(v_t[h][:, :, DV:DV + 1], 1.0)

                for pair in range(2):
                    for w_t, dst in ((wq_t, qT[pair]), (wk_t, kT[pair])):
                        for th in range(2):
                            ps = ppsum.tile([128, 1024], mybir.dt.float32, tag="qk_ps",
                                            name=f"qk_ps_{pair}_{th}")
                            for tq in range(2):
                                for dc in range(8):
                                    nc.tensor.matmul(
                                        ps[:, tq * 512:(tq + 1) * 512],
                                        w_t[:, dc, pair * 128:(pair + 1) * 128],
                                        ipT_t[:, dc, th * 1024 + tq * 512:th * 1024 + (tq + 1) * 512],
                                        start=(dc == 0), stop=(dc == 7))
                            nc.scalar.copy(dst[:, th * 1024:(th + 1) * 1024], ps[:])

                for tb in range(16):
                    psv = ppsum.tile([128, HLOC * DV], mybir.dt.float32, tag="v_ps",
                                     name=f"v_ps_{tb}")
                    for dc in range(8):
                        nc.tensor.matmul(psv[:],
                                         ipT_t[:, dc, tb * 128:(tb + 1) * 128],
                                         wv_t[:, dc, :],
                                         start=(dc == 0), stop=(dc == 7))
                    for h in range(HLOC):
                        nc.vector.tensor_copy(v_t[h][:, tb, 0:DV],
                                              psv[:, h * DV:(h + 1) * DV])

            # ---- attention: units = (head-pair, q-512-chunk), heads interleaved,
            #      1-bank score psums (bufs=3) so PE runs ahead of ACT exp;
            #      per-pair masked A2A fires as soon as the pair's ctx is done
            ctxf_half = []
            with tc.tile_pool(name="comm", bufs=1) as compool, \
                 tc.tile_pool(name="exp", bufs=48) as epool, \
                 tc.tile_pool(name="bc", bufs=3) as bcpool, \
                 tc.tile_pool(name="scA_ps", bufs=3, space="PSUM") as scpsA, \
                 tc.tile_pool(name="scB_ps", bufs=3, space="PSUM") as scpsB, \
                 tc.tile_pool(name="ctxA_ps", bufs=1, space="PSUM") as cxpsA, \
                 tc.tile_pool(name="ctxB_ps", bufs=1, space="PSUM") as cxpsB:
                for pair in range(2):
                    hA, hB = 2 * pair, 2 * pair + 1
                    for qc in range(4):
                        q0 = qc * 512
                        cpsA = cxpsA.tile([DV + 1, 512], mybir.dt.float32,
                                          tag="cpsA", name=f"cpsA_{pair}_{qc}")
                        cpsB = cxpsB.tile([DV + 1, 512], mybir.dt.float32,
                                          tag="cpsB", name=f"cpsB_{pair}_{qc}")
                        ekA, ekB = [], []
                        for kb in range(16):
                            k0 = kb * 128
                            spsA = scpsA.tile([128, 512], mybir.dt.float32,
                                              tag="spsA", name=f"spsA_{pair}_{qc}_{kb}")
                            spsB = scpsB.tile([128, 512], mybir.dt.float32,
                                              tag="spsB", name=f"spsB_{pair}_{qc}_{kb}")
                            nc.tensor.matmul(
                                spsA[:], kT[pair][0:64, k0:k0 + 128],
                                qT[pair][0:64, q0:q0 + 512],
                                start=True, stop=True, tile_position=(0, 0))
                            nc.tensor.matmul(
                                spsB[:], kT[pair][64:128, k0:k0 + 128],
                                qT[pair][64:128, q0:q0 + 512],
                                start=True, stop=True, tile_position=(64, 0))
                            eA = epool.tile([128, 512], bf16, tag="ekb",
                                            name=f"ekA_{pair}_{qc}_{kb}")
                            eB = epool.tile([128, 512], bf16, tag="ekb",
                                            name=f"ekB_{pair}_{qc}_{kb}")
                            ekA.append(eA)
                            ekB.append(eB)
                            nc.scalar.activation(eA[:], spsA[:], Act.Exp, scale=0.125)
                            nc.scalar.activation(eB[:], spsB[:], Act.Exp, scale=0.125)
                            nc.tensor.matmul(cpsA[:], v_t[hA][:, kb, :], eA[:],
                                             start=(kb == 0), stop=(kb == 15))
                            nc.tensor.matmul(cpsB[:], v_t[hB][:, kb, :], eB[:],
                                             start=(kb == 0), stop=(kb == 15))
                        for h, cps, ek in ((hA, cpsA, ekA), (hB, cpsB, ekB)):
                            # reciprocal of sums: spread [1,512] over 128
                            # partitions via DRAM so DVE divide is 4 elem/lane
                            sums_sb = bcpool.tile([DV + 1, 512], mybir.dt.float32,
                                                  tag="sums", name=f"sums_{h}_{qc}")
                            nc.vector.tensor_copy(sums_sb[DV:DV + 1, :],
                                                  cps[DV:DV + 1, :])
                            s_d = dpool.tile([1, 512], mybir.dt.float32, tag="s_d",
                                             name=f"s_d_{h}_{qc}")
                            nc.gpsimd.dma_start(s_d[:], sums_sb[DV:DV + 1, :])
                            sums_sp = bcpool.tile([128, 4], mybir.dt.float32,
                                                  tag="sums_sp", name=f"ssp_{h}_{qc}")
                            nc.gpsimd.dma_start(
                                sums_sp[:],
                                s_d[:].rearrange("o (p f) -> (o p) f", p=128))
                            rec_sp = bcpool.tile([128, 4], mybir.dt.float32,
                                                 tag="rec_sp", name=f"rsp_{h}_{qc}")
                            nc.vector.reciprocal(rec_sp[:], sums_sp[:])
                            rec_bf = bcpool.tile([128, 4], bf16, tag="rec_bf",
                                                 name=f"rbf_{h}_{qc}")
                            nc.vector.tensor_copy(rec_bf[:], rec_sp[:])
                            rb_d = dpool.tile([1, 512], bf16, tag="rb_d",
                                              name=f"rb_d_{h}_{qc}")
                            nc.gpsimd.dma_start(
                                rb_d[:].rearrange("o (p f) -> (o p) f", p=128),
                                rec_bf[:])
                            bcast = bcpool.tile([128, 512], bf16, tag="bcast",
                                                name=f"bcast_{h}_{qc}")
                            nc.gpsimd.dma_start(bcast[:],
                                              rb_d[:].to_broadcast((128, 512)))
                            # normalized context -> ctxT_loc (ch = h*64 + p)
                            cdst = ctxT_loc[:, h, q0:q0 + 512]
                            nc.vector.tensor_copy(cdst, cps[0:DV, :])
                            nc.vector.tensor_tensor(cdst, cdst, bcast[0:DV, :],
                                                    Alu.mult)
                            # normalized attention -> DRAM (bf16; host casts to f32)
                            for kb in range(16):
                                e = ek[kb]
                                nc.vector.tensor_tensor(e[:], e[:], bcast[:],
                                                        Alu.mult)
                                eng = nc.sync if kb % 2 == 0 else nc.scalar
                                eng.dma_start(
                                    attn_ext[h, kb * 128:(kb + 1) * 128,
                                             q0:q0 + 512],
                                    e[:])
                    # masked A2A for this pair's 128-channel half
                    staged = compool.tile([64, 2, 4, 2, 512], bf16, tag="staged",
                                          name=f"staged{pair}")
                    csrc = ctxT_loc[:, 2 * pair:2 * pair + 2, :] \
                        .rearrange("p h (c t) -> p c h t", t=512)
                    for d in range(2):
                        nc.vector.tensor_tensor(
                            staged[:, d], csrc[:],
                            mask_t[0:64, d * 4:(d + 1) * 4][:, :, None, None]
                                .to_broadcast((64, 4, 2, 512)),
                            Alu.mult)
                    a_in = dpool.tile([NCORES, 2, 64, 512], bf16,
                                      name=f"a2a_in{pair}")
                    a_out = dpool.tile([NCORES, 2, 64, 512], bf16,
                                       name=f"a2a_out{pair}")
                    nc.sync.dma_start(
                        a_in[:].rearrange("(d c) h p t -> p d c h t", d=2),
                        staged[:])
                    nc.gpsimd.collective_compute(
                        "AllToAll", Alu.bypass,
                        replica_groups=[list(range(NCORES))],
                        ins=[a_in.opt()], outs=[a_out.opt()])
                    rcv_lo = compool.tile([128, 4, 512], bf16, tag="rcvlo",
                                          name=f"rcvlo{pair}")
                    rcv_hi = compool.tile([128, 4, 512], bf16, tag="rcvhi",
                                          name=f"rcvhi{pair}")
                    nc.sync.dma_start(rcv_lo[:],
                                      a_out[0:4].rearrange("sl h p t -> (h p) sl t"))
                    nc.sync.dma_start(rcv_hi[:],
                                      a_out[4:8].rearrange("sl h p t -> (h p) sl t"))
                    cf = qkvpool.tile([128, 4, 512], bf16, name=f"ctxf{pair}")
                    nc.vector.tensor_tensor(cf[:], rcv_lo[:], rcv_hi[:], Alu.add)
                    ctxf_half.append(cf)

            # ---- tail: fc, residual, LayerNorm ----
            with tc.tile_pool(name="tail", bufs=1) as tpool, \
                 tc.tile_pool(name="ln", bufs=4) as lnpool, \
                 tc.tile_pool(name="sq", bufs=2) as sqpool, \
                 tc.tile_pool(name="fc_ps", bufs=2, space="PSUM") as fcpsum:
                ipres_t = tpool.tile([128, 4, D], mybir.dt.float32, name="ipres_t")
                fcb_t = tpool.tile([128, D], mybir.dt.float32, name="fcb_t")
                lng_t = tpool.tile([128, D], mybir.dt.float32, name="lng_t")
                lnb_t = tpool.tile([128, D], mybir.dt.float32, name="lnb_t")
                nc.sync.dma_start(ipres_t[:],
                                  ipres_ext.rearrange("(tb p) d -> p tb d", p=128))
                nc.sync.dma_start(fcb_t[:], fcb_ext[:])
                nc.sync.dma_start(lng_t[:], lng_ext[:])
                nc.sync.dma_start(lnb_t[:], lnb_ext[:])

                x_t = tpool.tile([128, 4, D], mybir.dt.float32, name="x_t")
                y_t = tpool.tile([128, 4, D], mybir.dt.float32, name="y_t")
                for tb in range(4):
                    fps = fcpsum.tile([128, D], mybir.dt.float32, tag="fps",
                                      name=f"fps_{tb}")
                    for n2 in range(2):
                        # global ch chunk cc = sl*2 + s  (s = pair half)
                        for i, (s, sl) in enumerate(
                                [(s, sl) for s in range(2) for sl in range(4)]):
                            nc.tensor.matmul(
                                fps[:, n2 * 512:(n2 + 1) * 512],
                                ctxf_half[s][:, sl, tb * 128:(tb + 1) * 128],
                                fcw_t[:, sl * 2 + s, n2 * 512:(n2 + 1) * 512],
                                start=(i == 0), stop=(i == 7))
                    xs = x_t[:, tb, :]
                    nc.vector.tensor_tensor(xs, fps[:], ipres_t[:, tb, :], Alu.add)
                    nc.vector.tensor_tensor(xs, xs, fcb_t[:], Alu.add)
                    # LayerNorm over D
                    ssum = lnpool.tile([128, 1], mybir.dt.float32, tag="ssum",
                                       name=f"ssum_{tb}")
                    nc.vector.tensor_reduce(ssum[:], xs, mybir.AxisListType.X, Alu.add)
                    nmu = lnpool.tile([128, 1], mybir.dt.float32, tag="nmu",
                                      name=f"nmu_{tb}")
                    nc.vector.tensor_scalar_mul(nmu[:], ssum[:], -1.0 / D)
                    sq = sqpool.tile([128, D], mybir.dt.float32, tag="sq",
                                     name=f"sq_{tb}")
                    ssq = lnpool.tile([128, 1], mybir.dt.float32, tag="ssq",
                                      name=f"ssq_{tb}")
                    nc.scalar.activation(sq[:], xs, Act.Square, bias=nmu[:],
                                         scale=1.0, accum_out=ssq[:])
                    veps = lnpool.tile([128, 1], mybir.dt.float32, tag="veps",
                                       name=f"veps_{tb}")
                    nc.vector.tensor_scalar(veps[:], ssq[:], 1.0 / D, EPS,
                                            Alu.mult, Alu.add)
                    lnv = lnpool.tile([128, 1], mybir.dt.float32, tag="lnv",
                                      name=f"lnv_{tb}")
                    nc.scalar.activation(lnv[:], veps[:], Act.Ln)
                    rstd = lnpool.tile([128, 1], mybir.dt.float32, tag="rstd",
                                       name=f"rstd_{tb}")
                    nc.scalar.activation(rstd[:], lnv[:], Act.Exp, scale=-0.5)
                    ys = y_t[:, tb, :]
                    nc.vector.tensor_scalar(ys, xs, nmu[:], rstd[:],
                                            Alu.add, Alu.mult)
                    nc.vector.tensor_tensor(ys, ys, lng_t[:], Alu.mult)
                    nc.vector.tensor_tensor(ys, ys, lnb_t[:], Alu.add)
                nc.sync.dma_start(y_ext.rearrange("(tb p) d -> p tb d", p=128), y_t[:])

    nc.finalize()
    return nc


def _prep_inputs(ip, wq, wk, wv, fc_w, fc_b, ln_g, ln_b):
    bf = ml_dtypes.bfloat16
    ip = np.asarray(ip, np.float32)
    wq = np.asarray(wq, np.float32)
    wk = np.asarray(wk, np.float32)
    wv = np.asarray(wv, np.float32)
    fc_w = np.asarray(fc_w, np.float32)
    fc_b = np.asarray(fc_b, np.float32)
    ln_g = np.asarray(ln_g, np.float32)
    ln_b = np.asarray(ln_b, np.float32)

    ipT = [np.ascontiguousarray(ip[b].T).astype(bf) for b in range(B)]
    fcw_bf = fc_w.astype(bf)
    fcb_bc = np.ascontiguousarray(np.broadcast_to(fc_b, (128, D))).astype(np.float32)
    lng_bc = np.ascontiguousarray(np.broadcast_to(ln_g, (128, D))).astype(np.float32)
    lnb_bc = np.ascontiguousarray(np.broadcast_to(ln_b, (128, D))).astype(np.float32)

    in_maps = []
    for c in range(NCORES):
        b, g = c // NCORES * 0 + c // 4, c % 4
        cols = slice(g * HLOC * DK, (g + 1) * HLOC * DK)
        mask = np.zeros((128, NCORES), np.float32)
        mask[:, b * 4:(b + 1) * 4] = 1.0
        in_maps.append({
            "ipT": ipT[b],
            "ip_res": np.ascontiguousarray(ip[b, g * TOK:(g + 1) * TOK]),
            "wq": np.ascontiguousarray(wq[:, cols]).astype(bf),
            "wk": np.ascontiguousarray(wk[:, cols]).astype(bf),
            "wv": np.ascontiguousarray(wv[:, cols]).astype(bf),
            "fc_w": fcw_bf,
            "fc_b_bc": fcb_bc,
            "ln_g_bc": lng_bc,
            "ln_b_bc": lnb_bc,
            "mask": mask.astype(bf),
        })
    return in_maps


def _run(in_maps, trace=False):
    from concourse.bass_utils import run_bass_kernel_spmd
    if "nc" not in _cache:
        _cache["nc"] = _build()
    return run_bass_kernel_spmd(_cache["nc"], in_maps,
                                core_ids=list(range(NCORES)), trace=trace)


def kernel(ip, wq, wk, wv, fc_w, fc_b, ln_g, ln_b, _trace=False):
    in_maps = _prep_inputs(ip, wq, wk, wv, fc_w, fc_b, ln_g, ln_b)
    res = _run(in_maps, trace=_trace)

    y = np.empty((B, S, D), np.float32)
    attn = np.empty((B, H, S, S), np.float32)
    for c in range(NCORES):
        b, g = c // 4, c % 4
        r = res.results[c]
        y[b, g * TOK:(g + 1) * TOK] = r["y_out"]
        for hl in range(HLOC):
            attn[b, g * HLOC + hl] = r["attn_out"][hl].T.astype(np.float32)
    if _trace:
        kernel.last_exec_time_ns = res.exec_time_ns
        kernel.last_results = res
    return y, attn


# revision 18
# speedup vs baseline: 1.1232x; 1.1065x over previous
"""Distributed Bass kernel for nn_AttentionLayer (B=2,S=2048,D=1024,H=16,DK=DV=64) on 8 TRN2 cores.

Sharding: core c handles batch c//4 and heads [(c%4)*4, (c%4)*4+4) (Megatron
column-sharded QKV).  Attention is computed with scores *transposed* ([k, q]
tiles, k on partitions) so the context matmul needs no on-chip transposes;
softmax row-sums come from a ones-column appended to V.  The attention
probability matrix is written to DRAM as attn^T per head (host re-transposes
during unshard).  The fc layer is token-parallel: a masked 8-core AllToAll
exchanges per-head context slices so each core computes fc+residual+LayerNorm
for its own 512-token slice of the full output.
"""
import sys

sys.path.insert(0, "/opt/trn_rl_repo")

import numpy as np
import ml_dtypes

B, S, D, H, DK, DV = 2, 2048, 1024, 16, 64, 64
NCORES = 8
HLOC = 4          # heads per core
TOK = 512         # tokens per core for the fc/LN output slice
EPS = 1e-6

_cache = {}


def _build():
    import concourse.bacc as bacc
    import concourse.tile as tile
    import concourse.mybir as mybir
    import concourse.hw_specs as hw_specs

    # Pin all ACT functions to the one table set containing Exp+Ln+Square+Copy
    # so the compiler never inserts mid-kernel table switches (~1.3us each).
    if not getattr(hw_specs, "_ant_tables_patched", False):
        _orig_tables = hw_specs.get_activation_tables

        def _single_set_tables(arch):
            t = dict(_orig_tables(arch))
            return {k: (v if k == "natural_log_exp_and_others" else set())
                    for k, v in t.items()}

        hw_specs.get_activation_tables = _single_set_tables
        hw_specs._ant_tables_patched = True
    import concourse.bacc as _b
    _b.get_activation_tables = hw_specs.get_activation_tables

    bf16 = mybir.dt.bfloat16
    f32 = mybir.dt.float32
    Act = mybir.ActivationFunctionType
    Alu = mybir.AluOpType

    nc = bacc.Bacc("TRN2", target_bir_lowering=False, debug=False,
                   num_devices=NCORES)

    ipT_ext = nc.dram_tensor("ipT", [D, S], bf16, kind="ExternalInput")
    ipres_ext = nc.dram_tensor("ip_res", [TOK, D], f32, kind="ExternalInput")
    wq_ext = nc.dram_tensor("wq", [D, HLOC * DK], bf16, kind="ExternalInput")
    wk_ext = nc.dram_tensor("wk", [D, HLOC * DK], bf16, kind="ExternalInput")
    wv_ext = nc.dram_tensor("wv", [D, HLOC * DV], bf16, kind="ExternalInput")
    fcw_ext = nc.dram_tensor("fc_w", [D, D], bf16, kind="ExternalInput")
    fcb_ext = nc.dram_tensor("fc_b_bc", [128, D], f32, kind="ExternalInput")
    lng_ext = nc.dram_tensor("ln_g_bc", [128, D], f32, kind="ExternalInput")
    lnb_ext = nc.dram_tensor("ln_b_bc", [128, D], f32, kind="ExternalInput")
    mask_ext = nc.dram_tensor("mask", [128, NCORES], bf16, kind="ExternalInput")

    attn_ext = nc.dram_tensor("attn_out", [HLOC, S, S], bf16, kind="ExternalOutput")
    y_ext = nc.dram_tensor("y_out", [TOK, D], f32, kind="ExternalOutput")

    with tile.TileContext(nc) as tc:
        with tc.tile_pool(name="const", bufs=1) as cpool, \
             tc.tile_pool(name="qkv", bufs=1) as qkvpool, \
             tc.tile_pool(name="dram", bufs=2, space="DRAM") as dpool:
            # ---- constant loads ----
            wq_t = cpool.tile([128, 8, HLOC * DK], bf16, name="wq_t")
            wk_t = cpool.tile([128, 8, HLOC * DK], bf16, name="wk_t")
            wv_t = cpool.tile([128, 8, HLOC * DV], bf16, name="wv_t")
            fcw_t = cpool.tile([128, 8, D], bf16, name="fcw_t")
            mask_t = cpool.tile([128, NCORES], bf16, name="mask_t")
            nc.sync.dma_start(wq_t[:], wq_ext.rearrange("(o i) c -> i o c", i=128))
            nc.sync.dma_start(wk_t[:], wk_ext.rearrange("(o i) c -> i o c", i=128))
            nc.sync.dma_start(wv_t[:], wv_ext.rearrange("(o i) c -> i o c", i=128))
            nc.sync.dma_start(mask_t[:], mask_ext[:])
            nc.sync.dma_start(fcw_t[:], fcw_ext.rearrange("(o i) c -> i o c", i=128))

            # qT/kT: channel-major per head-pair [128 = 2 heads x 64ch, S]
            qT = [qkvpool.tile([128, S], bf16, name=f"qT{p}") for p in range(2)]
            kT = [qkvpool.tile([128, S], bf16, name=f"kT{p}") for p in range(2)]
            # v~: token-major per head with ones column [128, 16, 65]
            v_t = [qkvpool.tile([128, 16, DV + 1], bf16, name=f"v{h}")
                   for h in range(HLOC)]
            # per-head context, channel-major: [64 dv partitions, head, token]
            ctxT_loc = qkvpool.tile([64, HLOC, S], bf16, name="ctxT_loc")

            # ---- projections ----
            with tc.tile_pool(name="ipt", bufs=1) as ipool, \
                 tc.tile_pool(name="proj_ps", bufs=2, space="PSUM") as ppsum:
                ipT_t = ipool.tile([128, 8, S], bf16, name="ipT_t")
                nc.sync.dma_start(ipT_t[:], ipT_ext.rearrange("(o i) t -> i o t", i=128))

                for h in range(HLOC):
                    nc.vector.memset(v_t[h][:, :, DV:DV + 1], 1.0)

                for pair in range(2):
                    for w_t, dst in ((wq_t, qT[pair]), (wk_t, kT[pair])):
                        for th in range(2):
                            ps = ppsum.tile([128, 1024], mybir.dt.float32, tag="qk_ps",
                                            name=f"qk_ps_{pair}_{th}")
                            for tq in range(2):
                                for dc in range(8):
                                    nc.tensor.matmul(
                                        ps[:, tq * 512:(tq + 1) * 512],
                                        w_t[:, dc, pair * 128:(pair + 1) * 128],
                                        ipT_t[:, dc, th * 1024 + tq * 512:th * 1024 + (tq + 1) * 512],
                                        start=(dc == 0), stop=(dc == 7))
                            nc.scalar.copy(dst[:, th * 1024:(th + 1) * 1024], ps[:])

                for tb in range(16):
                    psv = ppsum.tile([128, HLOC * DV], mybir.dt.float32, tag="v_ps",
                                     name=f"v_ps_{tb}")
                    for dc in range(8):
                        nc.tensor.matmul(psv[:],
                                         ipT_t[:, dc, tb * 128:(tb + 1) * 128],
                                         wv_t[:, dc, :],
                                         start=(dc == 0), stop=(dc == 7))
                    for h in range(HLOC):
                        nc.vector.tensor_copy(v_t[h][:, tb, 0:DV],
                                              psv[:, h * DV:(h + 1) * DV])

            # ---- attention: units = (head-pair, q-512-chunk), heads interleaved,
            #      1-bank score psums (bufs=3) so PE runs ahead of ACT exp;
            #      per-pair masked A2A fires as soon as the pair's ctx is done
            ctxf_half = []
            with tc.tile_pool(name="comm", bufs=1) as compool, \
                 tc.tile_pool(name="exp", bufs=64) as epool, \
                 tc.tile_pool(name="bc", bufs=3) as bcpool, \
                 tc.tile_pool(name="scA_ps", bufs=3, space="PSUM") as scpsA, \
                 tc.tile_pool(name="scB_ps", bufs=3, space="PSUM") as scpsB, \
                 tc.tile_pool(name="ctxA_ps", bufs=1, space="PSUM") as cxpsA, \
                 tc.tile_pool(name="ctxB_ps", bufs=1, space="PSUM") as cxpsB:
                for pair in range(2):
                    hA, hB = 2 * pair, 2 * pair + 1
                    for qc in range(4):
                        q0 = qc * 512
                        cpsA = cxpsA.tile([DV + 1, 512], mybir.dt.float32,
                                          tag="cpsA", name=f"cpsA_{pair}_{qc}")
                        cpsB = cxpsB.tile([DV + 1, 512], mybir.dt.float32,
                                          tag="cpsB", name=f"cpsB_{pair}_{qc}")
                        ekA, ekB = [], []
                        for kb in range(16):
                            k0 = kb * 128
                            spsA = scpsA.tile([128, 512], mybir.dt.float32,
                                              tag="spsA", name=f"spsA_{pair}_{qc}_{kb}")
                            spsB = scpsB.tile([128, 512], mybir.dt.float32,
                                              tag="spsB", name=f"spsB_{pair}_{qc}_{kb}")
                            nc.tensor.matmul(
                                spsA[:], kT[pair][0:64, k0:k0 + 128],
                                qT[pair][0:64, q0:q0 + 512],
                                start=True, stop=True, tile_position=(0, 0))
                            nc.tensor.matmul(
                                spsB[:], kT[pair][64:128, k0:k0 + 128],
                                qT[pair][64:128, q0:q0 + 512],
                                start=True, stop=True, tile_position=(64, 0))
                            eA = epool.tile([128, 512], bf16, tag="ekb",
                                            name=f"ekA_{pair}_{qc}_{kb}")
                            eB = epool.tile([128, 512], bf16, tag="ekb",
                                            name=f"ekB_{pair}_{qc}_{kb}")
                            ekA.append(eA)
                            ekB.append(eB)
                            nc.scalar.activation(eA[:], spsA[:], Act.Exp, scale=0.125)
                            nc.scalar.activation(eB[:], spsB[:], Act.Exp, scale=0.125)
                            nc.tensor.matmul(cpsA[:], v_t[hA][:, kb, :], eA[:],
                                             start=(kb == 0), stop=(kb == 15))
                            nc.tensor.matmul(cpsB[:], v_t[hB][:, kb, :], eB[:],
                                             start=(kb == 0), stop=(kb == 15))
                        # shared reciprocal chain for both heads: spread the
                        # two sums rows over 128 partitions via DRAM so the DVE
                        # divide is 8 elem/lane, then broadcast back per head
                        sums_sb = bcpool.tile([DV + 1, 2, 512], mybir.dt.float32,
                                              tag="sums", name=f"sums_{pair}_{qc}")
                        nc.vector.tensor_copy(sums_sb[DV:DV + 1, 0, :],
                                              cpsA[DV:DV + 1, :])
                        nc.vector.tensor_copy(sums_sb[DV:DV + 1, 1, :],
                                              cpsB[DV:DV + 1, :])
                        s_d = dpool.tile([2, 512], mybir.dt.float32, tag="s_d",
                                         name=f"s_d_{pair}_{qc}")
                        nc.gpsimd.dma_start(
                            s_d[:].rearrange("h t -> (h t)"),
                            sums_sb[DV:DV + 1, :, :].rearrange("o h t -> o (h t)"))
                        sums_sp = bcpool.tile([128, 8], mybir.dt.float32,
                                              tag="sums_sp", name=f"ssp_{pair}_{qc}")
                        nc.gpsimd.dma_start(
                            sums_sp[:],
                            s_d[:].rearrange("h t -> (h t)")
                                  .rearrange("(p f) -> p f", p=128))
                        rec_sp = bcpool.tile([128, 8], mybir.dt.float32,
                                             tag="rec_sp", name=f"rsp_{pair}_{qc}")
                        nc.vector.reciprocal(rec_sp[:], sums_sp[:])
                        rec_bf = bcpool.tile([128, 8], bf16, tag="rec_bf",
                                             name=f"rbf_{pair}_{qc}")
                        nc.vector.tensor_copy(rec_bf[:], rec_sp[:])
                        rb_d = dpool.tile([2, 512], bf16, tag="rb_d",
                                          name=f"rb_d_{pair}_{qc}")
                        nc.gpsimd.dma_start(
                            rb_d[:].rearrange("h t -> (h t)")
                                  .rearrange("(p f) -> p f", p=128),
                            rec_bf[:])
                        bcastA = bcpool.tile([128, 512], bf16, tag="bcastA",
                                             name=f"bcastA_{pair}_{qc}")
                        bcastB = bcpool.tile([128, 512], bf16, tag="bcastB",
                                             name=f"bcastB_{pair}_{qc}")
                        nc.gpsimd.dma_start(bcastA[:],
                                            rb_d[0:1, :].to_broadcast((128, 512)))
                        nc.gpsimd.dma_start(bcastB[:],
                                            rb_d[1:2, :].to_broadcast((128, 512)))
                        for h, cps, ek, bcast in ((hA, cpsA, ekA, bcastA),
                                                  (hB, cpsB, ekB, bcastB)):
                            # normalized context -> ctxT_loc (ch = h*64 + p)
                            cdst = ctxT_loc[:, h, q0:q0 + 512]
                            nc.vector.tensor_copy(cdst, cps[0:DV, :])
                            nc.vector.tensor_tensor(cdst, cdst, bcast[0:DV, :],
                                                    Alu.mult)
                            # normalized attention -> DRAM (bf16; host casts to f32)
                            for kb in range(16):
                                e = ek[kb]
                                nc.vector.tensor_tensor(e[:], e[:], bcast[:],
                                                        Alu.mult)
                                eng = nc.sync if kb % 2 == 0 else nc.scalar
                                eng.dma_start(
                                    attn_ext[h, kb * 128:(kb + 1) * 128,
                                             q0:q0 + 512],
                                    e[:])
                    # masked A2A for this pair's 128-channel half
                    staged = compool.tile([64, 2, 4, 2, 512], bf16, tag="staged",
                                          name=f"staged{pair}")
                    csrc = ctxT_loc[:, 2 * pair:2 * pair + 2, :] \
                        .rearrange("p h (c t) -> p c h t", t=512)
                    for d in range(2):
                        nc.vector.tensor_tensor(
                            staged[:, d], csrc[:],
                            mask_t[0:64, d * 4:(d + 1) * 4][:, :, None, None]
                                .to_broadcast((64, 4, 2, 512)),
                            Alu.mult)
                    a_in = dpool.tile([NCORES, 2, 64, 512], bf16,
                                      name=f"a2a_in{pair}")
                    a_out = dpool.tile([NCORES, 2, 64, 512], bf16,
                                       name=f"a2a_out{pair}")
                    nc.sync.dma_start(
                        a_in[:].rearrange("(d c) h p t -> p d c h t", d=2),
                        staged[:])
                    nc.gpsimd.collective_compute(
                        "AllToAll", Alu.bypass,
                        replica_groups=[list(range(NCORES))],
                        ins=[a_in.opt()], outs=[a_out.opt()])
                    rcv_lo = compool.tile([128, 4, 512], bf16, tag="rcvlo",
                                          name=f"rcvlo{pair}")
                    rcv_hi = compool.tile([128, 4, 512], bf16, tag="rcvhi",
                                          name=f"rcvhi{pair}")
                    nc.sync.dma_start(rcv_lo[:],
                                      a_out[0:4].rearrange("sl h p t -> (h p) sl t"))
                    nc.sync.dma_start(rcv_hi[:],
                                      a_out[4:8].rearrange("sl h p t -> (h p) sl t"))
                    cf = qkvpool.tile([128, 4, 512], bf16, name=f"ctxf{pair}")
                    nc.vector.tensor_tensor(cf[:], rcv_lo[:], rcv_hi[:], Alu.add)
                    ctxf_half.append(cf)

            # ---- tail: fc, residual, LayerNorm ----
            with tc.tile_pool(name="tail", bufs=1) as tpool, \
                 tc.tile_pool(name="ln", bufs=4) as lnpool, \
                 tc.tile_pool(name="sq", bufs=2) as sqpool, \
                 tc.tile_pool(name="fc_ps", bufs=2, space="PSUM") as fcpsum:
                ipres_t = tpool.tile([128, 4, D], mybir.dt.float32, name="ipres_t")
                fcb_t = tpool.tile([128, D], mybir.dt.float32, name="fcb_t")
                lng_t = tpool.tile([128, D], mybir.dt.float32, name="lng_t")
                lnb_t = tpool.tile([128, D], mybir.dt.float32, name="lnb_t")
                nc.sync.dma_start(ipres_t[:],
                                  ipres_ext.rearrange("(tb p) d -> p tb d", p=128))
                nc.sync.dma_start(fcb_t[:], fcb_ext[:])
                nc.sync.dma_start(lng_t[:], lng_ext[:])
                nc.sync.dma_start(lnb_t[:], lnb_ext[:])

                x_t = tpool.tile([128, 4, D], mybir.dt.float32, name="x_t")
                y_t = tpool.tile([128, 4, D], mybir.dt.float32, name="y_t")
                for tb in range(4):
                    fps = fcpsum.tile([128, D], mybir.dt.float32, tag="fps",
                                      name=f"fps_{tb}")
                    for n2 in range(2):
                        # global ch chunk cc = sl*2 + s  (s = pair half)
                        for i, (s, sl) in enumerate(
                                [(s, sl) for s in range(2) for sl in range(4)]):
                            nc.tensor.matmul(
                                fps[:, n2 * 512:(n2 + 1) * 512],
                                ctxf_half[s][:, sl, tb * 128:(tb + 1) * 128],
                                fcw_t[:, sl * 2 + s, n2 * 512:(n2 + 1) * 512],
                                start=(i == 0), stop=(i == 7))
                    xs = x_t[:, tb, :]
                    nc.vector.tensor_tensor(xs, fps[:], ipres_t[:, tb, :], Alu.add)
                    nc.vector.tensor_tensor(xs, xs, fcb_t[:], Alu.add)
                    # LayerNorm over D
                    ssum = lnpool.tile([128, 1], mybir.dt.float32, tag="ssum",
                                       name=f"ssum_{tb}")
                    nc.vector.tensor_reduce(ssum[:], xs, mybir.AxisListType.X, Alu.add)
                    nmu = lnpool.tile([128, 1], mybir.dt.float32, tag="nmu",
                                      name=f"nmu_{tb}")
                    nc.vector.tensor_scalar_mul(nmu[:], ssum[:], -1.0 / D)
                    sq = sqpool.tile([128, D], mybir.dt.float32, tag="sq",
                                     name=f"sq_{tb}")
                    ssq = lnpool.tile([128, 1], mybir.dt.float32, tag="ssq",
                                      name=f"ssq_{tb}")
                    nc.scalar.activation(sq[:], xs, Act.Square, bias=nmu[:],
                                         scale=1.0, accum_out=ssq[:])
                    veps = lnpool.tile([128, 1], mybir.dt.float32, tag="veps",
                                       name=f"veps_{tb}")
                    nc.vector.tensor_scalar(veps[:], ssq[:], 1.0 / D, EPS,
                                            Alu.mult, Alu.add)
                    lnv = lnpool.tile([128, 1], mybir.dt.float32, tag="lnv",
                                      name=f"lnv_{tb}")
                    nc.scalar.activation(lnv[:], veps[:], Act.Ln)
                    rstd = lnpool.tile([128, 1], mybir.dt.float32, tag="rstd",
                                       name=f"rstd_{tb}")
                    nc.scalar.activation(rstd[:], lnv[:], Act.Exp, scale=-0.5)
                    ys = y_t[:, tb, :]
                    nc.vector.tensor_scalar(ys, xs, nmu[:], rstd[:],
                                            Alu.add, Alu.mult)
                    nc.vector.tensor_tensor(ys, ys, lng_t[:], Alu.mult)
                    nc.vector.tensor_tensor(ys, ys, lnb_t[:], Alu.add)
                nc.sync.dma_start(y_ext.rearrange("(tb p) d -> p tb d", p=128), y_t[:])

    nc.finalize()
    return nc


def _prep_inputs(ip, wq, wk, wv, fc_w, fc_b, ln_g, ln_b):
    bf = ml_dtypes.bfloat16
    ip = np.asarray(ip, np.float32)
    wq = np.asarray(wq, np.float32)
    wk = np.asarray(wk, np.float32)
    wv = np.asarray(wv, np.float32)
    fc_w = np.asarray(fc_w, np.float32)
    fc_b = np.asarray(fc_b, np.float32)
    ln_g = np.asarray(ln_g, np.float32)
    ln_b = np.asarray(ln_b, np.float32)

    ipT = [np.ascontiguousarray(ip[b].T).astype(bf) for b in range(B)]
    fcw_bf = fc_w.astype(bf)
    fcb_bc = np.ascontiguousarray(np.broadcast_to(fc_b, (128, D))).astype(np.float32)
    lng_bc = np.ascontiguousarray(np.broadcast_to(ln_g, (128, D))).astype(np.float32)
    lnb_bc = np.ascontiguousarray(np.broadcast_to(ln_b, (128, D))).astype(np.float32)

    in_maps = []
    for c in range(NCORES):
        b, g = c // NCORES * 0 + c // 4, c % 4
        cols = slice(g * HLOC * DK, (g + 1) * HLOC * DK)
        mask = np.zeros((128, NCORES), np.float32)
        mask[:, b * 4:(b + 1) * 4] = 1.0
        in_maps.append({
            "ipT": ipT[b],
            "ip_res": np.ascontiguousarray(ip[b, g * TOK:(g + 1) * TOK]),
            "wq": np.ascontiguousarray(wq[:, cols]).astype(bf),
            "wk": np.ascontiguousarray(wk[:, cols]).astype(bf),
            "wv": np.ascontiguousarray(wv[:, cols]).astype(bf),
            "fc_w": fcw_bf,
            "fc_b_bc": fcb_bc,
            "ln_g_bc": lng_bc,
            "ln_b_bc": lnb_bc,
            "mask": mask.astype(bf),
        })
    return in_maps


def _run(in_maps, trace=False):
    from concourse.bass_utils import run_bass_kernel_spmd
    if "nc" not in _cache:
        _cache["nc"] = _build()
    return run_bass_kernel_spmd(_cache["nc"], in_maps,
                                core_ids=list(range(NCORES)), trace=trace)


def kernel(ip, wq, wk, wv, fc_w, fc_b, ln_g, ln_b, _trace=False):
    in_maps = _prep_inputs(ip, wq, wk, wv, fc_w, fc_b, ln_g, ln_b)
    res = _run(in_maps, trace=_trace)

    y = np.empty((B, S, D), np.float32)
    attn = np.empty((B, H, S, S), np.float32)
    for c in range(NCORES):
        b, g = c // 4, c % 4
        r = res.results[c]
        y[b, g * TOK:(g + 1) * TOK] = r["y_out"]
        for hl in range(HLOC):
            attn[b, g * HLOC + hl] = r["attn_out"][hl].T.astype(np.float32)
    if _trace:
        kernel.last_exec_time_ns = res.exec_time_ns
        kernel.last_results = res
    return y, attn


# revision 19
# speedup vs baseline: 1.1939x; 1.0630x over previous
"""Distributed Bass kernel for nn_AttentionLayer (B=2,S=2048,D=1024,H=16,DK=DV=64) on 8 TRN2 cores.

Sharding: core c handles batch c//4 and heads [(c%4)*4, (c%4)*4+4) (Megatron
column-sharded QKV).  Attention is computed with scores *transposed* ([k, q]
tiles, k on partitions) so the context matmul needs no on-chip transposes;
softmax row-sums come from a ones-column appended to V.  The attention
probability matrix is written to DRAM as attn^T per head (host re-transposes
during unshard).  The fc layer is token-parallel: a masked 8-core AllToAll
exchanges per-head context slices so each core computes fc+residual+LayerNorm
for its own 512-token slice of the full output.
"""
import sys

sys.path.insert(0, "/opt/trn_rl_repo")

import numpy as np
import ml_dtypes

B, S, D, H, DK, DV = 2, 2048, 1024, 16, 64, 64
NCORES = 8
HLOC = 4          # heads per core
TOK = 512         # tokens per core for the fc/LN output slice
EPS = 1e-6

_cache = {}


def _build():
    import concourse.bacc as bacc
    import concourse.tile as tile
    import concourse.mybir as mybir
    import concourse.hw_specs as hw_specs

    # Pin all ACT functions to the one table set containing Exp+Ln+Square+Copy
    # so the compiler never inserts mid-kernel table switches (~1.3us each).
    if not getattr(hw_specs, "_ant_tables_patched", False):
        _orig_tables = hw_specs.get_activation_tables

        def _single_set_tables(arch):
            t = dict(_orig_tables(arch))
            return {k: (v if k == "natural_log_exp_and_others" else set())
                    for k, v in t.items()}

        hw_specs.get_activation_tables = _single_set_tables
        hw_specs._ant_tables_patched = True
    import concourse.bacc as _b
    _b.get_activation_tables = hw_specs.get_activation_tables

    bf16 = mybir.dt.bfloat16
    f32 = mybir.dt.float32
    Act = mybir.ActivationFunctionType
    Alu = mybir.AluOpType

    nc = bacc.Bacc("TRN2", target_bir_lowering=False, debug=False,
                   num_devices=NCORES)

    ipT_ext = nc.dram_tensor("ipT", [D, S], bf16, kind="ExternalInput")
    ipres_ext = nc.dram_tensor("ip_res", [TOK, D], f32, kind="ExternalInput")
    wq_ext = nc.dram_tensor("wq", [D, HLOC * DK], bf16, kind="ExternalInput")
    wk_ext = nc.dram_tensor("wk", [D, HLOC * DK], bf16, kind="ExternalInput")
    wv_ext = nc.dram_tensor("wv", [D, HLOC * DV], bf16, kind="ExternalInput")
    fcw_ext = nc.dram_tensor("fc_w", [D, D], bf16, kind="ExternalInput")
    fcb_ext = nc.dram_tensor("fc_b_bc", [128, D], f32, kind="ExternalInput")
    lng_ext = nc.dram_tensor("ln_g_bc", [128, D], f32, kind="ExternalInput")
    lnb_ext = nc.dram_tensor("ln_b_bc", [128, D], f32, kind="ExternalInput")
    mask_ext = nc.dram_tensor("mask", [128, NCORES], bf16, kind="ExternalInput")

    attn_ext = nc.dram_tensor("attn_out", [HLOC, S, S], bf16, kind="ExternalOutput")
    y_ext = nc.dram_tensor("y_out", [TOK, D], f32, kind="ExternalOutput")

    with tile.TileContext(nc) as tc:
        with tc.tile_pool(name="const", bufs=1) as cpool, \
             tc.tile_pool(name="qkv", bufs=1) as qkvpool, \
             tc.tile_pool(name="dram", bufs=2, space="DRAM") as dpool:
            # ---- constant loads ----
            wq_t = cpool.tile([128, 8, HLOC * DK], bf16, name="wq_t")
            wk_t = cpool.tile([128, 8, HLOC * DK], bf16, name="wk_t")
            wv_t = cpool.tile([128, 8, HLOC * DV], bf16, name="wv_t")
            fcw_t = cpool.tile([128, 8, D], bf16, name="fcw_t")
            mask_t = cpool.tile([128, NCORES], bf16, name="mask_t")
            nc.sync.dma_start(wq_t[:], wq_ext.rearrange("(o i) c -> i o c", i=128))
            nc.sync.dma_start(wk_t[:], wk_ext.rearrange("(o i) c -> i o c", i=128))
            nc.sync.dma_start(wv_t[:], wv_ext.rearrange("(o i) c -> i o c", i=128))
            nc.sync.dma_start(mask_t[:], mask_ext[:])
            nc.sync.dma_start(fcw_t[:], fcw_ext.rearrange("(o i) c -> i o c", i=128))

            # qT/kT: channel-major per head-pair [128 = 2 heads x 64ch, S]
            qT = [qkvpool.tile([128, S], bf16, name=f"qT{p}") for p in range(2)]
            kT = [qkvpool.tile([128, S], bf16, name=f"kT{p}") for p in range(2)]
            # v~: token-major per head with ones column [128, 16, 65]
            v_t = [qkvpool.tile([128, 16, DV + 1], bf16, name=f"v{h}")
                   for h in range(HLOC)]
            # per-head context, channel-major: [64 dv partitions, head, token]
            ctxT_loc = qkvpool.tile([64, HLOC, S], bf16, name="ctxT_loc")

            # ---- projections ----
            with tc.tile_pool(name="ipt", bufs=1) as ipool, \
                 tc.tile_pool(name="proj_ps", bufs=2, space="PSUM") as ppsum:
                ipT_t = ipool.tile([128, 8, S], bf16, name="ipT_t")
                for th2 in range(2):
                    nc.sync.dma_start(
                        ipT_t[:, :, th2 * 1024:(th2 + 1) * 1024],
                        ipT_ext[:, th2 * 1024:(th2 + 1) * 1024]
                            .rearrange("(o i) t -> i o t", i=128))

                for h in range(HLOC):
                    nc.vector.memset(v_t[h][:, :, DV:DV + 1], 1.0)

                for pair in range(2):
                    for w_t, dst in ((wq_t, qT[pair]), (wk_t, kT[pair])):
                        for th in range(2):
                            ps = ppsum.tile([128, 1024], mybir.dt.float32, tag="qk_ps",
                                            name=f"qk_ps_{pair}_{th}")
                            for tq in range(2):
                                for dc in range(8):
                                    nc.tensor.matmul(
                                        ps[:, tq * 512:(tq + 1) * 512],
                                        w_t[:, dc, pair * 128:(pair + 1) * 128],
                                        ipT_t[:, dc, th * 1024 + tq * 512:th * 1024 + (tq + 1) * 512],
                                        start=(dc == 0), stop=(dc == 7))
                            nc.scalar.copy(dst[:, th * 1024:(th + 1) * 1024], ps[:])

                for tb in range(16):
                    psv = ppsum.tile([128, HLOC * DV], mybir.dt.float32, tag="v_ps",
                                     name=f"v_ps_{tb}")
                    for dc in range(8):
                        nc.tensor.matmul(psv[:],
                                         ipT_t[:, dc, tb * 128:(tb + 1) * 128],
                                         wv_t[:, dc, :],
                                         start=(dc == 0), stop=(dc == 7))
                    for h in range(HLOC):
                        nc.vector.tensor_copy(v_t[h][:, tb, 0:DV],
                                              psv[:, h * DV:(h + 1) * DV])

            # ---- attention: units = (head-pair, q-512-chunk), heads interleaved,
            #      1-bank score psums (bufs=3) so PE runs ahead of ACT exp;
            #      per-pair masked A2A fires as soon as the pair's ctx is done
            ctxf_half = []
            with tc.tile_pool(name="comm", bufs=1) as compool, \
                 tc.tile_pool(name="exp", bufs=64) as epool, \
                 tc.tile_pool(name="bc", bufs=3) as bcpool, \
                 tc.tile_pool(name="scA_ps", bufs=3, space="PSUM") as scpsA, \
                 tc.tile_pool(name="scB_ps", bufs=3, space="PSUM") as scpsB, \
                 tc.tile_pool(name="ctxA_ps", bufs=1, space="PSUM") as cxpsA, \
                 tc.tile_pool(name="ctxB_ps", bufs=1, space="PSUM") as cxpsB:
                for pair in range(2):
                    hA, hB = 2 * pair, 2 * pair + 1
                    for qc in range(4):
                        q0 = qc * 512
                        cpsA = cxpsA.tile([DV + 1, 512], mybir.dt.float32,
                                          tag="cpsA", name=f"cpsA_{pair}_{qc}")
                        cpsB = cxpsB.tile([DV + 1, 512], mybir.dt.float32,
                                          tag="cpsB", name=f"cpsB_{pair}_{qc}")
                        ekA, ekB = [], []
                        for kb in range(16):
                            k0 = kb * 128
                            spsA = scpsA.tile([128, 512], mybir.dt.float32,
                                              tag="spsA", name=f"spsA_{pair}_{qc}_{kb}")
                            spsB = scpsB.tile([128, 512], mybir.dt.float32,
                                              tag="spsB", name=f"spsB_{pair}_{qc}_{kb}")
                            nc.tensor.matmul(
                                spsA[:], kT[pair][0:64, k0:k0 + 128],
                                qT[pair][0:64, q0:q0 + 512],
                                start=True, stop=True, tile_position=(0, 0))
                            nc.tensor.matmul(
                                spsB[:], kT[pair][64:128, k0:k0 + 128],
                                qT[pair][64:128, q0:q0 + 512],
                                start=True, stop=True, tile_position=(64, 0))
                            eA = epool.tile([128, 512], bf16, tag="ekb",
                                            name=f"ekA_{pair}_{qc}_{kb}")
                            eB = epool.tile([128, 512], bf16, tag="ekb",
                                            name=f"ekB_{pair}_{qc}_{kb}")
                            ekA.append(eA)
                            ekB.append(eB)
                            nc.scalar.activation(eA[:], spsA[:], Act.Exp, scale=0.125)
                            nc.scalar.activation(eB[:], spsB[:], Act.Exp, scale=0.125)
                            nc.tensor.matmul(cpsA[:], v_t[hA][:, kb, :], eA[:],
                                             start=(kb == 0), stop=(kb == 15))
                            nc.tensor.matmul(cpsB[:], v_t[hB][:, kb, :], eB[:],
                                             start=(kb == 0), stop=(kb == 15))
                        # shared reciprocal chain for both heads: spread the
                        # two sums rows over 128 partitions via DRAM so the DVE
                        # divide is 8 elem/lane, then broadcast back per head
                        sums_sb = bcpool.tile([DV + 1, 2, 512], mybir.dt.float32,
                                              tag="sums", name=f"sums_{pair}_{qc}")
                        nc.vector.tensor_copy(sums_sb[DV:DV + 1, 0, :],
                                              cpsA[DV:DV + 1, :])
                        nc.vector.tensor_copy(sums_sb[DV:DV + 1, 1, :],
                                              cpsB[DV:DV + 1, :])
                        ceng = nc.gpsimd if pair == 0 else nc.scalar
                        s_d = dpool.tile([2, 512], mybir.dt.float32, tag="s_d",
                                         name=f"s_d_{pair}_{qc}")
                        ceng.dma_start(
                            s_d[:].rearrange("h t -> (h t)"),
                            sums_sb[DV:DV + 1, :, :].rearrange("o h t -> o (h t)"))
                        sums_sp = bcpool.tile([128, 8], mybir.dt.float32,
                                              tag="sums_sp", name=f"ssp_{pair}_{qc}")
                        ceng.dma_start(
                            sums_sp[:],
                            s_d[:].rearrange("h t -> (h t)")
                                  .rearrange("(p f) -> p f", p=128))
                        rec_sp = bcpool.tile([128, 8], mybir.dt.float32,
                                             tag="rec_sp", name=f"rsp_{pair}_{qc}")
                        nc.vector.reciprocal(rec_sp[:], sums_sp[:])
                        rec_bf = bcpool.tile([128, 8], bf16, tag="rec_bf",
                                             name=f"rbf_{pair}_{qc}")
                        nc.vector.tensor_copy(rec_bf[:], rec_sp[:])
                        rb_d = dpool.tile([2, 512], bf16, tag="rb_d",
                                          name=f"rb_d_{pair}_{qc}")
                        ceng.dma_start(
                            rb_d[:].rearrange("h t -> (h t)")
                                  .rearrange("(p f) -> p f", p=128),
                            rec_bf[:])
                        bcastA = bcpool.tile([128, 512], bf16, tag="bcastA",
                                             name=f"bcastA_{pair}_{qc}")
                        bcastB = bcpool.tile([128, 512], bf16, tag="bcastB",
                                             name=f"bcastB_{pair}_{qc}")
                        ceng.dma_start(bcastA[:],
                                       rb_d[0:1, :].to_broadcast((128, 512)))
                        ceng.dma_start(bcastB[:],
                                       rb_d[1:2, :].to_broadcast((128, 512)))
                        for h, cps, ek, bcast in ((hA, cpsA, ekA, bcastA),
                                                  (hB, cpsB, ekB, bcastB)):
                            # normalized context -> ctxT_loc (ch = h*64 + p)
                            cdst = ctxT_loc[:, h, q0:q0 + 512]
                            nc.vector.tensor_copy(cdst, cps[0:DV, :])
                            nc.vector.tensor_tensor(cdst, cdst, bcast[0:DV, :],
                                                    Alu.mult)
                            # normalized attention -> DRAM (bf16; host casts to f32)
                            for kb in range(16):
                                e = ek[kb]
                                nc.vector.tensor_tensor(e[:], e[:], bcast[:],
                                                        Alu.mult)
                                if kb % 2 == 0:
                                    eng = nc.sync
                                else:
                                    eng = nc.scalar if pair == 0 else nc.gpsimd
                                eng.dma_start(
                                    attn_ext[h, kb * 128:(kb + 1) * 128,
                                             q0:q0 + 512],
                                    e[:])
                    # masked A2A for this pair's 128-channel half
                    staged = compool.tile([64, 2, 4, 2, 512], bf16, tag="staged",
                                          name=f"staged{pair}")
                    csrc = ctxT_loc[:, 2 * pair:2 * pair + 2, :] \
                        .rearrange("p h (c t) -> p c h t", t=512)
                    for d in range(2):
                        nc.vector.tensor_tensor(
                            staged[:, d], csrc[:],
                            mask_t[0:64, d * 4:(d + 1) * 4][:, :, None, None]
                                .to_broadcast((64, 4, 2, 512)),
                            Alu.mult)
                    a_in = dpool.tile([NCORES, 2, 64, 512], bf16,
                                      name=f"a2a_in{pair}")
                    a_out = dpool.tile([NCORES, 2, 64, 512], bf16,
                                       name=f"a2a_out{pair}")
                    nc.sync.dma_start(
                        a_in[:].rearrange("(d c) h p t -> p d c h t", d=2),
                        staged[:])
                    nc.gpsimd.collective_compute(
                        "AllToAll", Alu.bypass,
                        replica_groups=[list(range(NCORES))],
                        ins=[a_in.opt()], outs=[a_out.opt()])
                    rcv_lo = compool.tile([128, 4, 512], bf16, tag="rcvlo",
                                          name=f"rcvlo{pair}")
                    rcv_hi = compool.tile([128, 4, 512], bf16, tag="rcvhi",
                                          name=f"rcvhi{pair}")
                    nc.sync.dma_start(rcv_lo[:],
                                      a_out[0:4].rearrange("sl h p t -> (h p) sl t"))
                    nc.sync.dma_start(rcv_hi[:],
                                      a_out[4:8].rearrange("sl h p t -> (h p) sl t"))
                    cf = qkvpool.tile([128, 4, 512], bf16, name=f"ctxf{pair}")
                    nc.vector.tensor_tensor(cf[:], rcv_lo[:], rcv_hi[:], Alu.add)
                    ctxf_half.append(cf)

            # ---- tail: fc, residual, LayerNorm ----
            with tc.tile_pool(name="tail", bufs=1) as tpool, \
                 tc.tile_pool(name="ln", bufs=4) as lnpool, \
                 tc.tile_pool(name="sq", bufs=2) as sqpool, \
                 tc.tile_pool(name="fc_ps", bufs=2, space="PSUM") as fcpsum:
                ipres_t = tpool.tile([128, 4, D], mybir.dt.float32, name="ipres_t")
                fcb_t = tpool.tile([128, D], mybir.dt.float32, name="fcb_t")
                lng_t = tpool.tile([128, D], mybir.dt.float32, name="lng_t")
                lnb_t = tpool.tile([128, D], mybir.dt.float32, name="lnb_t")
                nc.sync.dma_start(ipres_t[:],
                                  ipres_ext.rearrange("(tb p) d -> p tb d", p=128))
                nc.sync.dma_start(fcb_t[:], fcb_ext[:])
                nc.sync.dma_start(lng_t[:], lng_ext[:])
                nc.sync.dma_start(lnb_t[:], lnb_ext[:])

                x_t = tpool.tile([128, 4, D], mybir.dt.float32, name="x_t")
                y_t = tpool.tile([128, 4, D], mybir.dt.float32, name="y_t")
                for tb in range(4):
                    fps = fcpsum.tile([128, D], mybir.dt.float32, tag="fps",
                                      name=f"fps_{tb}")
                    for n2 in range(2):
                        # global ch chunk cc = sl*2 + s  (s = pair half)
                        for i, (s, sl) in enumerate(
                                [(s, sl) for s in range(2) for sl in range(4)]):
                            nc.tensor.matmul(
                                fps[:, n2 * 512:(n2 + 1) * 512],
                                ctxf_half[s][:, sl, tb * 128:(tb + 1) * 128],
                                fcw_t[:, sl * 2 + s, n2 * 512:(n2 + 1) * 512],
                                start=(i == 0), stop=(i == 7))
                    xs = x_t[:, tb, :]
                    nc.vector.tensor_tensor(xs, fps[:], ipres_t[:, tb, :], Alu.add)
                    nc.vector.tensor_tensor(xs, xs, fcb_t[:], Alu.add)
                    # LayerNorm over D
                    ssum = lnpool.tile([128, 1], mybir.dt.float32, tag="ssum",
                                       name=f"ssum_{tb}")
                    nc.vector.tensor_reduce(ssum[:], xs, mybir.AxisListType.X, Alu.add)
                    nmu = lnpool.tile([128, 1], mybir.dt.float32, tag="nmu",
                                      name=f"nmu_{tb}")
                    nc.vector.tensor_scalar_mul(nmu[:], ssum[:], -1.0 / D)
                    sq = sqpool.tile([128, D], mybir.dt.float32, tag="sq",
                                     name=f"sq_{tb}")
                    ssq = lnpool.tile([128, 1], mybir.dt.float32, tag="ssq",
                                      name=f"ssq_{tb}")
                    nc.scalar.activation(sq[:], xs, Act.Square, bias=nmu[:],
                                         scale=1.0, accum_out=ssq[:])
                    veps = lnpool.tile([128, 1], mybir.dt.float32, tag="veps",
                                       name=f"veps_{tb}")
                    nc.vector.tensor_scalar(veps[:], ssq[:], 1.0 / D, EPS,
                                            Alu.mult, Alu.add)
                    lnv = lnpool.tile([128, 1], mybir.dt.float32, tag="lnv",
                                      name=f"lnv_{tb}")
                    nc.scalar.activation(lnv[:], veps[:], Act.Ln)
                    rstd = lnpool.tile([128, 1], mybir.dt.float32, tag="rstd",
                                       name=f"rstd_{tb}")
                    nc.scalar.activation(rstd[:], lnv[:], Act.Exp, scale=-0.5)
                    ys = y_t[:, tb, :]
                    nc.vector.tensor_scalar(ys, xs, nmu[:], rstd[:],
                                            Alu.add, Alu.mult)
                    nc.vector.tensor_tensor(ys, ys, lng_t[:], Alu.mult)
                    nc.vector.tensor_tensor(ys, ys, lnb_t[:], Alu.add)
                nc.sync.dma_start(y_ext.rearrange("(tb p) d -> p tb d", p=128), y_t[:])

    nc.finalize()
    return nc


def _prep_inputs(ip, wq, wk, wv, fc_w, fc_b, ln_g, ln_b):
    bf = ml_dtypes.bfloat16
    ip = np.asarray(ip, np.float32)
    wq = np.asarray(wq, np.float32)
    wk = np.asarray(wk, np.float32)
    wv = np.asarray(wv, np.float32)
    fc_w = np.asarray(fc_w, np.float32)
    fc_b = np.asarray(fc_b, np.float32)
    ln_g = np.asarray(ln_g, np.float32)
    ln_b = np.asarray(ln_b, np.float32)

    ipT = [np.ascontiguousarray(ip[b].T).astype(bf) for b in range(B)]
    fcw_bf = fc_w.astype(bf)
    fcb_bc = np.ascontiguousarray(np.broadcast_to(fc_b, (128, D))).astype(np.float32)
    lng_bc = np.ascontiguousarray(np.broadcast_to(ln_g, (128, D))).astype(np.float32)
    lnb_bc = np.ascontiguousarray(np.broadcast_to(ln_b, (128, D))).astype(np.float32)

    in_maps = []
    for c in range(NCORES):
        b, g = c // NCORES * 0 + c // 4, c % 4
        cols = slice(g * HLOC * DK, (g + 1) * HLOC * DK)
        mask = np.zeros((128, NCORES), np.float32)
        mask[:, b * 4:(b + 1) * 4] = 1.0
        in_maps.append({
            "ipT": ipT[b],
            "ip_res": np.ascontiguousarray(ip[b, g * TOK:(g + 1) * TOK]),
            "wq": np.ascontiguousarray(wq[:, cols]).astype(bf),
            "wk": np.ascontiguousarray(wk[:, cols]).astype(bf),
            "wv": np.ascontiguousarray(wv[:, cols]).astype(bf),
            "fc_w": fcw_bf,
            "fc_b_bc": fcb_bc,
            "ln_g_bc": lng_bc,
            "ln_b_bc": lnb_bc,
            "mask": mask.astype(bf),
        })
    return in_maps


def _run(in_maps, trace=False):
    from concourse.bass_utils import run_bass_kernel_spmd
    if "nc" not in _cache:
        _cache["nc"] = _build()
    return run_bass_kernel_spmd(_cache["nc"], in_maps,
                                core_ids=list(range(NCORES)), trace=trace)


def kernel(ip, wq, wk, wv, fc_w, fc_b, ln_g, ln_b, _trace=False):
    in_maps = _prep_inputs(ip, wq, wk, wv, fc_w, fc_b, ln_g, ln_b)
    res = _run(in_maps, trace=_trace)

    y = np.empty((B, S, D), np.float32)
    attn = np.empty((B, H, S, S), np.float32)
    for c in range(NCORES):
        b, g = c // 4, c % 4
        r = res.results[c]
        y[b, g * TOK:(g + 1) * TOK] = r["y_out"]
        for hl in range(HLOC):
            attn[b, g * HLOC + hl] = r["attn_out"][hl].T.astype(np.float32)
    if _trace:
        kernel.last_exec_time_ns = res.exec_time_ns
        kernel.last_results = res
    return y, attn
